# revision 10
# baseline (speedup 1.0000x reference)
"""Conv2D (N=32, Cin=128, 56x56 -> Cout=256, 3x3, pad 1, stride 1) on 8 Trainium2
NeuronCores.

Strategy: data-parallel over batch (4 images per core), conv lowered to 9
shifted matmuls (one per filter tap) accumulating in PSUM over the
Cin=128-partition contraction dim.  Cout=256 is handled as 2 halves of 128
output partitions.  bf16 operands: same 1 column/cycle PE streaming as
fp32r, but FWL halves LDWEIGHTS (~100 ns < 193 ns matmul) so the per-tap
weight reload never gates the stream; scale-rel error ~2.5e-3.

Loop order is tap-OUTER: for each (image, cout-half), each tap's stationary
weight walks all 7 row-blocks (7 PSUM banks).  The 8th bank is used at
startup by a warm-up accumulation chain (8 x N=512 matmuls on scratch) that
keeps the PE busy from ~1.5us so the HAM clock gate opens (1.2 -> 2.4 GHz)
right as the first real matmul issues.

DMA layout: whole-image input in 2 chunks (rows 0-34 / 32-58) on the sync
queue; all 9 tap weights as one [128, 2304] tile in 2 DMAs on the vector
queue (tap0 first so the first LDW isn't gated); output written PADDED
(58-wide rows) to DRAM scratch [img, cout, rb, 464], in rb-PAIR fused DMAs
(1856B-per-partition contiguous descriptors) split across the sync and
scalar queues; the 2 garbage columns per row are sliced out on the host.
PSUM drains (bias-add + copy to SBUF) alternate Vector/Scalar engines.
"""

import os
import sys

import numpy as np

sys.path.insert(0, "/opt/trn_rl_repo")

import concourse.tile as tile
from concourse import bacc, mybir

N, CIN, H, W = 32, 128, 56, 56
COUT, KH, KW = 256, 3, 3
NCORES = 8
NPER = N // NCORES  # images per core
HP, WP = H + 3, W + 2  # padded spatial dims (1 top + 2 bottom, 1 left + 1 right)
FLAT = HP * WP  # 3422 padded pixels per image per cin
RB = 8  # output rows per PSUM chunk
NRB = H // RB  # 7 row-blocks per image
CHUNK = RB * WP  # 464 <= 512 fp32 PSUM bank limit
NTAP = KH * KW

# input tile split: xa covers padded rows [0, 35) for rbs 0-3,
# xb covers padded rows [32, 59) for rbs 4-6
XA_ROWS = 35
XB_ROW0 = 32
XA_LEN = XA_ROWS * WP  # 2030
XB_LEN = (HP - XB_ROW0) * WP  # 27*58 = 1566

NWARM = 8  # warm-up accumulation-chain matmuls (N=512 each, ~3.4us cold)

MM_MODE = os.environ.get("CONV_MM_MODE", "bf16")

_CACHE = {}


def _build(mm_mode):
    f32 = mybir.dt.float32
    in_dt = {
        "fp32": f32,
        "fp32r": mybir.dt.float32r,
        "bf16": mybir.dt.bfloat16,
    }[mm_mode]

    nc = bacc.Bacc(None, target_bir_lowering=False)
    xp_d = nc.declare_dram_parameter("xp", [NPER, CIN, FLAT], in_dt, isOutput=False)
    w_d = nc.declare_dram_parameter("w", [CIN, NTAP * COUT], in_dt, isOutput=False)
    b_d = nc.declare_dram_parameter("b", [CIN, 2], f32, isOutput=False)
    # padded output: [img, cout, rb, 8*58]; garbage cols sliced on host
    y_d = nc.declare_dram_parameter("y", [NPER, COUT, NRB, CHUNK], f32, isOutput=True)

    with tile.TileContext(nc) as tc:
        with (
            tc.tile_pool(name="xa", bufs=2) as xapool,
            tc.tile_pool(name="xb", bufs=2) as xbpool,
            tc.tile_pool(name="wgt", bufs=1) as wpool,
            tc.tile_pool(name="bias", bufs=1) as bpool,
            tc.tile_pool(name="out", bufs=8) as opool,
            tc.tile_pool(name="ps", bufs=7, space="PSUM") as pspool,
            tc.tile_pool(name="warm", bufs=1, space="PSUM") as warmpool,
        ):
            # all 9 tap weights in one tile; tap0's slice goes first on the
            # sync queue (so the first LDW isn't gated), the rest + bias on
            # the scalar queue
            w_sb = wpool.tile([CIN, NTAP * COUT], in_dt)
            nc.sync.dma_start(out=w_sb[:, 0:COUT], in_=w_d[:, 0:COUT])
            nc.scalar.dma_start(
                out=w_sb[:, COUT : NTAP * COUT], in_=w_d[:, COUT : NTAP * COUT]
            )
            b_sb = bpool.tile([CIN, 2], f32)
            nc.scalar.dma_start(out=b_sb[:], in_=b_d[:, :])

            # Warm-up: accumulation chain on scratch SBUF into one PSUM bank.
            # Back-to-back streaming keeps the PE fully busy (no WAR gaps)
            # so the HAM clock gate opens before the first real matmul.
            warm_x = bpool.tile([CIN, 512], in_dt, tag="warm")
            nc.gpsimd.memset(warm_x[:], 0)
            wps = warmpool.tile([128, 512], f32)
            for wi in range(NWARM):
                nc.tensor.matmul(
                    wps[:],
                    warm_x[:, 0:128],
                    warm_x[:],
                    start=(wi == 0),
                    stop=(wi == NWARM - 1),
                )

            for i in range(NPER):
                xa = xapool.tile([CIN, XA_LEN], in_dt, tag="xa")
                nc.sync.dma_start(out=xa[:], in_=xp_d[i, :, 0:XA_LEN])
                xb = xbpool.tile([CIN, XB_LEN], in_dt, tag="xb")
                nc.sync.dma_start(
                    out=xb[:], in_=xp_d[i, :, XB_ROW0 * WP : XB_ROW0 * WP + XB_LEN]
                )
                for half in range(2):
                    ps_tiles = [
                        pspool.tile(
                            [128, CHUNK], f32, name=f"ps_{i}_{half}_{rb}", tag="ps"
                        )
                        for rb in range(NRB)
                    ]
                    for tap in range(NTAP):
                        kh, kw = divmod(tap, KW)
                        wsl = w_sb[:, tap * COUT + half * 128 : tap * COUT + half * 128 + 128]
                        for rb in range(NRB):
                            off = (rb * RB + kh) * WP + kw
                            if rb < 4:
                                src = xa[:, off : off + CHUNK]
                            else:
                                o = off - XB_ROW0 * WP
                                src = xb[:, o : o + CHUNK]
                            nc.tensor.matmul(
                                ps_tiles[rb][:],
                                wsl,
                                src,
                                start=(tap == 0),
                                stop=(tap == NTAP - 1),
                            )
                    # drain in rb pairs into shared tiles -> fused output DMAs
                    for pair in range(3):
                        r0 = 2 * pair
                        ot = opool.tile(
                            [128, 2, CHUNK], f32, name=f"ot_{i}_{half}_{pair}", tag="ot"
                        )
                        nc.vector.tensor_scalar_add(
                            ot[:, 0, :], ps_tiles[r0][:], b_sb[:, half : half + 1]
                        )
                        nc.scalar.activation(
                            ot[:, 1, :],
                            ps_tiles[r0 + 1][:],
                            mybir.ActivationFunctionType.Identity,
                            bias=b_sb[:, half : half + 1],
                        )
                        dma_eng = nc.sync if pair % 2 == 0 else nc.scalar
                        dma_eng.dma_start(
                            out=y_d[i, half * 128 : half * 128 + 128, r0 : r0 + 2, :],
                            in_=ot[:],
                        )
                    ot6 = opool.tile([128, CHUNK], f32, name=f"ot6_{i}_{half}", tag="ot6")
                    nc.vector.tensor_scalar_add(
                        ot6[:], ps_tiles[6][:], b_sb[:, half : half + 1]
                    )
                    nc.scalar.dma_start(
                        out=y_d[i, half * 128 : half * 128 + 128, 6, :], in_=ot6[:]
                    )
    nc.finalize()
    return nc


def get_nc(mm_mode=None):
    mm_mode = mm_mode or MM_MODE
    if mm_mode not in _CACHE:
        _CACHE[mm_mode] = _build(mm_mode)
    return _CACHE[mm_mode]


def _round_fp32r(a):
    """Round fp32 array to the fp32r grid (8-bit exp, 11-bit mantissa, top 20
    bits of the word) with round-to-nearest so the PE's truncation of the low
    12 bits lands on the nearest representable value."""
    u = np.ascontiguousarray(a, np.float32).view(np.uint32)
    u = u + 0x7FF + ((u >> 12) & 1)
    u &= np.uint32(0xFFFFF000)
    return u.view(np.float32)


def prep_inputs(x, weight, bias, mm_mode=None):
    """Host-side staging: zero-pad x to 59x58 and flatten, retile weights to
    [cin, tap*cout], split per-core input maps."""
    mm_mode = mm_mode or MM_MODE
    x = np.asarray(x, np.float32)
    weight = np.asarray(weight, np.float32)
    bias = np.asarray(bias, np.float32)

    xp = np.zeros((N, CIN, HP, WP), np.float32)
    xp[:, :, 1 : H + 1, 1 : W + 1] = x
    # [cout, cin, kh, kw] -> [cin, tap*cout]
    w_prep = np.ascontiguousarray(
        weight.transpose(1, 2, 3, 0).reshape(CIN, NTAP * COUT)
    )
    if mm_mode == "bf16":
        import ml_dtypes

        xp = xp.astype(ml_dtypes.bfloat16)
        w_prep = w_prep.astype(ml_dtypes.bfloat16)
    elif mm_mode == "fp32r":
        xp = _round_fp32r(xp)
        w_prep = _round_fp32r(w_prep)
    xp = xp.reshape(N, CIN, FLAT)
    b_prep = np.ascontiguousarray(bias.reshape(2, 128).T.astype(np.float32))

    return [
        {
            "xp": np.ascontiguousarray(xp[c * NPER : (c + 1) * NPER]),
            "w": w_prep,
            "b": b_prep,
        }
        for c in range(NCORES)
    ]


def _unpad_output(y_pad):
    """[NPER, COUT, NRB, 464] padded rows -> [NPER, COUT, 56, 56]."""
    y = y_pad.reshape(NPER, COUT, NRB, RB, WP)[:, :, :, :, :W]
    return np.ascontiguousarray(y.reshape(NPER, COUT, H, W))


def kernel(x, weight, bias, mm_mode=None, trace=False, tmpdir=None):
    from concourse.bass_utils import run_bass_kernel_spmd

    nc = get_nc(mm_mode)
    in_maps = prep_inputs(x, weight, bias, mm_mode)
    res = run_bass_kernel_spmd(
        nc, in_maps, list(range(NCORES)), trace=trace, tmpdir=tmpdir
    )
    out = np.concatenate([_unpad_output(r["y"]) for r in res.results], axis=0)
    if trace:
        kernel.last_results = res
    return out
